# revision 6
# baseline (speedup 1.0000x reference)
"""Bahdanau additive-attention pooling for Trainium2 (Bass/Tile).

Reference math (per batch):
    q = x @ Wt; k = x @ Wx                                  [L, U]
    e[i,j] = sum_u Wa[u] * tanh(q[i,u] + k[j,u] + bh[u])    (+ ba, dropped --
                                                             softmax shift-inv)
    v = softmax_j(e) @ x                                    [L, D]

Sharding: 8 cores = 4 batches x 2 query-halves (data-parallel, no
collectives).  Per core: 512 queries x 1024 keys, flash-style over query
blocks of 128 so the [L, L, U] tensor h is never materialized.

Per-core layout: partitions p = 32*uu + ii, where ii indexes 32 queries of a
"group" and uu 4 of the 32 u's; u-slices us = 0..7 cover u = 4*us+uu.  Groups
are query-strided (group g = queries {16*ii + g}) so every cross-partition
data movement is a clean strided DMA; the output DMA un-permutes.

  K4[us][p, j] = k[j, 4us+uu]      PE matmul, host-replicated Wx4, fp32r
  Qb[us][p, g] = q[16ii+g, ...]+bh qT -> DRAM -> strided gather-back
  S  = K4[us] + Qb[us][:, g]       VectorE tensor_scalar (2x fp32 mode)
  H  = tanh(S)                     ScalarE, batched 4 u-slices per instr,
                                   fp16 output (the engine bottleneck:
                                   L*L*U/8 = 16.8M lanes-elems per core)
  e[32c:32c+32, :] += wa32[us].T@H PE, M=32 col-tiled at partition base 32c
                                   (fp16: full rate + legal dst partition;
                                   fp32r is full-rate but base-0 only),
                                   8 accumulating matmuls contract u
  P = exp(e)                       ScalarE on the [128q, 1024k] PSUM block,
                                   row-sums via accum_out (|e| <= ~4.5, so
                                   no max-subtraction is needed)
  aT chunks = PE transpose(P); v = sum_jc aT[jc].T @ x[jc] (fp32r); scale by
  1/rowsum on VectorE; DMA out.

Engine budget per core (model): ScalarE ~131us (85% busy - bound by the
16.8M-element tanh volume at 1 elem/cycle/lane @1.2GHz), VectorE ~86us,
PE ~76us, total ~153us.
"""

import numpy as np

import concourse.bass as bass
import concourse.mybir as mybir
import concourse.tile as tile
from concourse import bacc
from concourse.bass import ds, ts

B, L, D, U = 4, 1024, 256, 32
NCORES = 8
HALVES = 2
LQ = L // HALVES                # 512 queries per core
GQ = 32                         # queries per group
NGRP = LQ // GQ                 # 16 groups
NUS = 8                         # u-slices (4 u's each)
USB = 4                         # u-slices per tanh batch
QB = 128                        # query block (softmax granularity)
NQB = LQ // QB                  # 4
NJC = L // 128                  # 8 key chunks
NDC = D // 128                  # 2 contraction chunks

F32 = mybir.dt.float32
F32R = mybir.dt.float32r
F16 = mybir.dt.float16
AF = mybir.ActivationFunctionType


def build_kernel(nc: bass.Bass):
    x_d = nc.dram_tensor("x", [L, D], F32R, kind="ExternalInput")
    xq_d = nc.dram_tensor("xq", [LQ, D], F32R, kind="ExternalInput")
    wt_d = nc.dram_tensor("wt", [D, U], F32R, kind="ExternalInput")
    wx4_d = nc.dram_tensor("wx4", [D, NUS, 128], F32R, kind="ExternalInput")
    wa32_d = nc.dram_tensor("wa32", [NUS, 128, GQ], F16, kind="ExternalInput")
    bh_d = nc.dram_tensor("bh", [U, 1], F32, kind="ExternalInput")
    ident_d = nc.dram_tensor("ident", [128, 128], F32R, kind="ExternalInput")
    out_d = nc.dram_tensor("out", [LQ, D], F32, kind="ExternalOutput")
    qtb_d = nc.dram_tensor("qtb", [U, LQ], F32)  # scratch for the Qb gather

    with tile.TileContext(nc) as tc:
        with tc.tile_pool(name="const", bufs=1) as cpool:
            x_sb = cpool.tile([128, NJC, D], F32R)
            xq_sb = cpool.tile([128, NQB, D], F32R)
            xT_sb = cpool.tile([128, NDC, L], F32R)
            xqT_sb = cpool.tile([128, NDC, LQ], F32R)
            wt_sb = cpool.tile([128, NDC, U], F32R)
            wx4_sb = cpool.tile([128, NDC, NUS, 128], F32R)
            wa32_sb = cpool.tile([128, NUS, GQ], F16)
            bh_sb = cpool.tile([U, 1], F32)
            ident_sb = cpool.tile([128, 128], F32R)
            k4_sb = cpool.tile([128, NUS, L], F32)
            qtb_sb = cpool.tile([U, LQ], F32)
            qb_sb = cpool.tile([128, NUS, NGRP], F32)
            sums_sb = cpool.tile([128, NQB], F32)
            recip_sb = cpool.tile([128, NQB], F32)

            # small/critical DMAs first; 1MB wx4 split per-us and last
            nc.sync.dma_start(ident_sb[:], ident_d.ap())
            nc.sync.dma_start(bh_sb[:], bh_d.ap())
            nc.sync.dma_start(wt_sb[:], wt_d.ap().rearrange("(c p) u -> p c u", p=128))
            nc.sync.dma_start(wa32_sb[:], wa32_d.ap().rearrange("us p m -> p us m"))
            xq_r = xq_d.ap().rearrange("(c p) d -> c p d", p=128)
            for jc in range(NQB):
                nc.sync.dma_start(xq_sb[:, jc, :], xq_r[jc])
            x_r = x_d.ap().rearrange("(c p) d -> c p d", p=128)
            for jc in range(NJC):
                eng = nc.sync if jc % 2 == 0 else nc.gpsimd
                eng.dma_start(x_sb[:, jc, :], x_r[jc])
            wx4_r = wx4_d.ap().rearrange("(c p) us m -> p c us m", p=128)
            for us in range(NUS):
                nc.gpsimd.dma_start(wx4_sb[:, :, us, :], wx4_r[:, :, us, :])

            # ---- prologue ----
            with (
                tc.tile_pool(name="ptr", bufs=3, space="PSUM") as ptr,
                tc.tile_pool(name="pk4", bufs=2, space="PSUM") as pk4,
                tc.tile_pool(name="pqt", bufs=1, space="PSUM") as pqt,
            ):
                # xq^T first: the qT -> DRAM -> gather chain is the longest
                for dc in range(NDC):
                    tr4 = ptr.tile([128, 512], F32R)
                    for jc in range(NQB):
                        nc.tensor.transpose(
                            tr4[:, ts(jc, 128)],
                            xq_sb[:, jc, ds(dc * 128, 128)],
                            ident_sb[:],
                        )
                    nc.scalar.copy(xqT_sb[:, dc, :], tr4[:])
                qt_ps = pqt.tile([U, LQ], F32)
                for dc in range(NDC):
                    nc.tensor.matmul(
                        qt_ps[:],
                        wt_sb[:, dc, :],
                        xqT_sb[:, dc, :],
                        start=(dc == 0),
                        stop=(dc == NDC - 1),
                    )
                nc.vector.tensor_scalar_add(qtb_sb[:], qt_ps[:], bh_sb[:])
                nc.sync.dma_start(qtb_d.ap(), qtb_sb[:])
                # Qb[us][32uu+ii, g] = qtb[4us+uu, 16ii+g]  (strided groups:
                # group g holds queries {16ii+g}) -> contiguous 64B runs
                qtb_r = qtb_d.ap().rearrange(
                    "(us uu) (ii g) -> uu ii us g", uu=4, g=NGRP
                )
                for uu in range(4):
                    dst = qb_sb[ds(32 * uu, GQ), :, :]
                    nc.sync.dma_start(dst, qtb_r[uu])

                # x^T: 4 chunk-transposes per PSUM tile, one copy per tile
                for n in range(L // 512):
                    for dc in range(NDC):
                        tr4 = ptr.tile([128, 512], F32R)
                        for q4 in range(4):
                            jc = 4 * n + q4
                            nc.tensor.transpose(
                                tr4[:, ts(q4, 128)],
                                x_sb[:, jc, ds(dc * 128, 128)],
                                ident_sb[:],
                            )
                        nc.vector.tensor_copy(
                            xT_sb[:, dc, ds(n * 512, 512)], tr4[:]
                        )

                # K4[us] = k^T slice-replicated, via host-replicated Wx4
                for us in range(NUS):
                    kp = pk4.tile([128, L], F32)
                    for n in range(L // 512):
                        for dc in range(NDC):
                            nc.tensor.matmul(
                                kp[:, ds(n * 512, 512)],
                                wx4_sb[:, dc, us, :],
                                xT_sb[:, dc, ds(n * 512, 512)],
                                start=(dc == 0),
                                stop=(dc == NDC - 1),
                            )
                    nc.scalar.copy(k4_sb[:, us, :], kp[:])

            # ---- main loop ----
            with (
                tc.tile_pool(name="spool", bufs=3) as spool,
                tc.tile_pool(name="hpool", bufs=3) as hpool,
                tc.tile_pool(name="ppool", bufs=2) as ppool,
                tc.tile_pool(name="atpool", bufs=2) as atpool,
                tc.tile_pool(name="vpool", bufs=2) as vpool,
                tc.tile_pool(name="pe", bufs=2, space="PSUM") as pe_e,
                tc.tile_pool(name="pat", bufs=1, space="PSUM") as pe_at,
                tc.tile_pool(name="pv", bufs=2, space="PSUM") as pe_v,
            ):
                out_r = out_d.ap().rearrange(
                    "(ii gg c) d -> gg c ii d", gg=NQB, c=4
                )
                for qb in range(NQB):
                    e_ps = pe_e.tile([128, L], F32)
                    for c in range(4):
                        g = 4 * qb + c
                        for b2 in range(NUS // USB):
                            s = spool.tile([128, USB, L], F32)
                            for k in range(USB):
                                us = USB * b2 + k
                                nc.vector.tensor_scalar_add(
                                    s[:, k, :],
                                    k4_sb[:, us, :],
                                    qb_sb[:, us, ds(g, 1)],
                                )
                            h = hpool.tile([128, USB, L], F16)
                            nc.scalar.activation(h[:], s[:], AF.Tanh)
                            for k in range(USB):
                                us = USB * b2 + k
                                for n in range(L // 512):
                                    nc.tensor.matmul(
                                        e_ps[ds(32 * c, 32), ds(n * 512, 512)],
                                        wa32_sb[:, us, :],
                                        h[:, k, ds(n * 512, 512)],
                                        start=(us == 0),
                                        stop=(us == NUS - 1),
                                        tile_position=(0, 32 * c),
                                    )
                    p = ppool.tile([128, L], F32R)
                    nc.scalar.activation(
                        p[:], e_ps[:], AF.Exp, accum_out=sums_sb[:, ds(qb, 1)]
                    )
                    nc.vector.reciprocal(recip_sb[:, ds(qb, 1)], sums_sb[:, ds(qb, 1)])
                    at_sb = atpool.tile([128, NJC, 128], F32R)
                    at_ps = pe_at.tile([128, L], F32R)
                    for jc in range(NJC):
                        nc.tensor.transpose(
                            at_ps[:, ts(jc, 128)], p[:, ts(jc, 128)], ident_sb[:]
                        )
                    nc.vector.tensor_copy(at_sb[:], at_ps[:])
                    v_ps = pe_v.tile([128, D], F32)
                    for jc in range(NJC):
                        nc.tensor.matmul(
                            v_ps[:],
                            at_sb[:, jc, :],
                            x_sb[:, jc, :],
                            start=(jc == 0),
                            stop=(jc == NJC - 1),
                        )
                    v_sb = vpool.tile([128, D], F32)
                    nc.vector.tensor_scalar_mul(
                        v_sb[:], v_ps[:], recip_sb[:, ds(qb, 1)]
                    )
                    nc.sync.dma_start(out_r[qb], v_sb[:])

    return nc


_NC_CACHE: dict = {}


def get_compiled_nc():
    if "nc" not in _NC_CACHE:
        nc = bacc.Bacc("TRN2", target_bir_lowering=False, debug=False)
        build_kernel(nc)
        nc.compile()
        _NC_CACHE["nc"] = nc
    return _NC_CACHE["nc"]


def make_in_maps(inputs_np, Wt, Wx, bh, Wa):
    wx4 = np.zeros((D, NUS, 128), np.float32)
    wa32 = np.zeros((NUS, 128, GQ), np.float16)
    for us in range(NUS):
        for uu in range(4):
            u = 4 * us + uu
            wx4[:, us, 32 * uu : 32 * (uu + 1)] = Wx[:, u : u + 1]
            wa32[us, 32 * uu : 32 * (uu + 1), :] = Wa[u, 0] * np.eye(GQ, dtype=np.float32)
    bh_c = bh.reshape(U, 1).astype(np.float32)
    ident = np.eye(128, dtype=np.float32)
    in_maps = []
    for c in range(NCORES):
        b, half = divmod(c, HALVES)
        in_maps.append(
            {
                "x": np.ascontiguousarray(inputs_np[b]),
                "xq": np.ascontiguousarray(inputs_np[b, half * LQ : (half + 1) * LQ]),
                "wt": Wt,
                "wx4": wx4,
                "wa32": wa32,
                "bh": bh_c,
                "ident": ident,
            }
        )
    return in_maps


def kernel(**inputs) -> np.ndarray:
    x = np.asarray(inputs["inputs"], dtype=np.float32)
    Wt = np.ascontiguousarray(np.asarray(inputs["Wt"], np.float32))
    Wx = np.ascontiguousarray(np.asarray(inputs["Wx"], np.float32))
    bh = np.asarray(inputs["bh"], np.float32)
    Wa = np.asarray(inputs["Wa"], np.float32)

    from concourse.bass_utils import run_bass_kernel_spmd

    nc = get_compiled_nc()
    in_maps = make_in_maps(x, Wt, Wx, bh, Wa)
    res = run_bass_kernel_spmd(nc, in_maps, list(range(NCORES)))
    kernel._last_results = res  # type: ignore[attr-defined]

    out = np.empty((B, L, D), np.float32)
    for c in range(NCORES):
        b, half = divmod(c, HALVES)
        out[b, half * LQ : (half + 1) * LQ] = res.results[c]["out"]
    return out


# revision 7
# speedup vs baseline: 1.0045x; 1.0045x over previous
"""Bahdanau additive-attention pooling for Trainium2 (Bass/Tile).

Reference math (per batch):
    q = x @ Wt; k = x @ Wx                                  [L, U]
    e[i,j] = sum_u Wa[u] * tanh(q[i,u] + k[j,u] + bh[u])    (+ ba, dropped --
                                                             softmax shift-inv)
    v = softmax_j(e) @ x                                    [L, D]

Sharding: 8 cores = 4 batches x 2 query-halves (data-parallel, no
collectives).  Per core: 512 queries x 1024 keys, flash-style over query
blocks of 128 so the [L, L, U] tensor h is never materialized.

Per-core layout: partitions p = 32*uu + ii, where ii indexes 32 queries of a
"group" and uu 4 of the 32 u's; u-slices us = 0..7 cover u = 4*us+uu.  Groups
are query-strided (group g = queries {16*ii + g}) so every cross-partition
data movement is a clean strided DMA; the output DMA un-permutes.

  K4[us][p, j] = k[j, 4us+uu]      PE matmul, host-replicated Wx4, fp32r
  Qb[us][p, g] = q[16ii+g, ...]+bh qT -> DRAM -> strided gather-back
  S  = K4[us] + Qb[us][:, g]       VectorE tensor_scalar; K4 and S are fp16
                                   (16-bit packed DVE mode, ~2x; halves the
                                   K4 PSUM->SBUF copy payload on ScalarE)
  H  = tanh(S)                     ScalarE, batched 4 u-slices per instr,
                                   fp16 output (the engine bottleneck:
                                   L*L*U/8 = 16.8M lanes-elems per core)
  e[32c:32c+32, :] += wa32[us].T@H PE, M=32 col-tiled at partition base 32c
                                   (fp16: full rate + legal dst partition;
                                   fp32r is full-rate but base-0 only),
                                   8 accumulating matmuls contract u
  P = exp(e)                       ScalarE on the [128q, 1024k] PSUM block,
                                   row-sums via accum_out (|e| <= ~4.5, so
                                   no max-subtraction is needed)
  aT chunks = PE transpose(P); v = sum_jc aT[jc].T @ x[jc] (fp32r); scale by
  1/rowsum on VectorE; DMA out.

Engine budget per core (model): ScalarE ~131us (86% busy - bound by the
16.8M-element tanh volume at 1 elem/cycle/lane @1.2GHz), PE ~76us,
VectorE ~51us, total ~152us.
"""

import numpy as np

import concourse.bass as bass
import concourse.mybir as mybir
import concourse.tile as tile
from concourse import bacc
from concourse.bass import ds, ts

B, L, D, U = 4, 1024, 256, 32
NCORES = 8
HALVES = 2
LQ = L // HALVES                # 512 queries per core
GQ = 32                         # queries per group
NGRP = LQ // GQ                 # 16 groups
NUS = 8                         # u-slices (4 u's each)
USB = 4                         # u-slices per tanh batch
QB = 128                        # query block (softmax granularity)
NQB = LQ // QB                  # 4
NJC = L // 128                  # 8 key chunks
NDC = D // 128                  # 2 contraction chunks

F32 = mybir.dt.float32
F32R = mybir.dt.float32r
F16 = mybir.dt.float16
AF = mybir.ActivationFunctionType


def build_kernel(nc: bass.Bass):
    x_d = nc.dram_tensor("x", [L, D], F32R, kind="ExternalInput")
    xq_d = nc.dram_tensor("xq", [LQ, D], F32R, kind="ExternalInput")
    wt_d = nc.dram_tensor("wt", [D, U], F32R, kind="ExternalInput")
    wx4_d = nc.dram_tensor("wx4", [D, NUS, 128], F32R, kind="ExternalInput")
    wa32_d = nc.dram_tensor("wa32", [NUS, 128, GQ], F16, kind="ExternalInput")
    bh_d = nc.dram_tensor("bh", [U, 1], F32, kind="ExternalInput")
    ident_d = nc.dram_tensor("ident", [128, 128], F32R, kind="ExternalInput")
    out_d = nc.dram_tensor("out", [LQ, D], F32, kind="ExternalOutput")
    qtb_d = nc.dram_tensor("qtb", [U, LQ], F32)  # scratch for the Qb gather

    with tile.TileContext(nc) as tc:
        with tc.tile_pool(name="const", bufs=1) as cpool:
            x_sb = cpool.tile([128, NJC, D], F32R)
            xq_sb = cpool.tile([128, NQB, D], F32R)
            xT_sb = cpool.tile([128, NDC, L], F32R)
            xqT_sb = cpool.tile([128, NDC, LQ], F32R)
            wt_sb = cpool.tile([128, NDC, U], F32R)
            wx4_sb = cpool.tile([128, NDC, NUS, 128], F32R)
            wa32_sb = cpool.tile([128, NUS, GQ], F16)
            bh_sb = cpool.tile([U, 1], F32)
            ident_sb = cpool.tile([128, 128], F32R)
            k4_sb = cpool.tile([128, NUS, L], F16)
            qtb_sb = cpool.tile([U, LQ], F32)
            qb_sb = cpool.tile([128, NUS, NGRP], F32)
            sums_sb = cpool.tile([128, NQB], F32)
            recip_sb = cpool.tile([128, NQB], F32)

            # small/critical DMAs first; 1MB wx4 split per-us and last
            nc.sync.dma_start(ident_sb[:], ident_d.ap())
            nc.sync.dma_start(bh_sb[:], bh_d.ap())
            nc.sync.dma_start(wt_sb[:], wt_d.ap().rearrange("(c p) u -> p c u", p=128))
            nc.sync.dma_start(wa32_sb[:], wa32_d.ap().rearrange("us p m -> p us m"))
            xq_r = xq_d.ap().rearrange("(c p) d -> c p d", p=128)
            for jc in range(NQB):
                nc.sync.dma_start(xq_sb[:, jc, :], xq_r[jc])
            x_r = x_d.ap().rearrange("(c p) d -> c p d", p=128)
            for jc in range(NJC):
                eng = nc.sync if jc % 2 == 0 else nc.gpsimd
                eng.dma_start(x_sb[:, jc, :], x_r[jc])
            wx4_r = wx4_d.ap().rearrange("(c p) us m -> p c us m", p=128)
            for us in range(NUS):
                nc.gpsimd.dma_start(wx4_sb[:, :, us, :], wx4_r[:, :, us, :])

            # ---- prologue ----
            with (
                tc.tile_pool(name="ptr", bufs=3, space="PSUM") as ptr,
                tc.tile_pool(name="pk4", bufs=2, space="PSUM") as pk4,
                tc.tile_pool(name="pqt", bufs=1, space="PSUM") as pqt,
            ):
                # xq^T first: the qT -> DRAM -> gather chain is the longest
                for dc in range(NDC):
                    tr4 = ptr.tile([128, 512], F32R)
                    for jc in range(NQB):
                        nc.tensor.transpose(
                            tr4[:, ts(jc, 128)],
                            xq_sb[:, jc, ds(dc * 128, 128)],
                            ident_sb[:],
                        )
                    nc.scalar.copy(xqT_sb[:, dc, :], tr4[:])
                qt_ps = pqt.tile([U, LQ], F32)
                for dc in range(NDC):
                    nc.tensor.matmul(
                        qt_ps[:],
                        wt_sb[:, dc, :],
                        xqT_sb[:, dc, :],
                        start=(dc == 0),
                        stop=(dc == NDC - 1),
                    )
                nc.vector.tensor_scalar_add(qtb_sb[:], qt_ps[:], bh_sb[:])
                nc.sync.dma_start(qtb_d.ap(), qtb_sb[:])
                # Qb[us][32uu+ii, g] = qtb[4us+uu, 16ii+g]  (strided groups:
                # group g holds queries {16ii+g}) -> contiguous 64B runs
                qtb_r = qtb_d.ap().rearrange(
                    "(us uu) (ii g) -> uu ii us g", uu=4, g=NGRP
                )
                for uu in range(4):
                    dst = qb_sb[ds(32 * uu, GQ), :, :]
                    nc.sync.dma_start(dst, qtb_r[uu])

                # x^T: 4 chunk-transposes per PSUM tile, one copy per tile
                for n in range(L // 512):
                    for dc in range(NDC):
                        tr4 = ptr.tile([128, 512], F32R)
                        for q4 in range(4):
                            jc = 4 * n + q4
                            nc.tensor.transpose(
                                tr4[:, ts(q4, 128)],
                                x_sb[:, jc, ds(dc * 128, 128)],
                                ident_sb[:],
                            )
                        nc.vector.tensor_copy(
                            xT_sb[:, dc, ds(n * 512, 512)], tr4[:]
                        )

                # K4[us] = k^T slice-replicated, via host-replicated Wx4
                for us in range(NUS):
                    kp = pk4.tile([128, L], F32)
                    for n in range(L // 512):
                        for dc in range(NDC):
                            nc.tensor.matmul(
                                kp[:, ds(n * 512, 512)],
                                wx4_sb[:, dc, us, :],
                                xT_sb[:, dc, ds(n * 512, 512)],
                                start=(dc == 0),
                                stop=(dc == NDC - 1),
                            )
                    nc.scalar.copy(k4_sb[:, us, :], kp[:])

            # ---- main loop ----
            with (
                tc.tile_pool(name="spool", bufs=3) as spool,
                tc.tile_pool(name="hpool", bufs=3) as hpool,
                tc.tile_pool(name="ppool", bufs=2) as ppool,
                tc.tile_pool(name="atpool", bufs=2) as atpool,
                tc.tile_pool(name="vpool", bufs=2) as vpool,
                tc.tile_pool(name="pe", bufs=2, space="PSUM") as pe_e,
                tc.tile_pool(name="pat", bufs=1, space="PSUM") as pe_at,
                tc.tile_pool(name="pv", bufs=2, space="PSUM") as pe_v,
            ):
                out_r = out_d.ap().rearrange(
                    "(ii gg c) d -> gg c ii d", gg=NQB, c=4
                )
                for qb in range(NQB):
                    e_ps = pe_e.tile([128, L], F32)
                    for c in range(4):
                        g = 4 * qb + c
                        for b2 in range(NUS // USB):
                            s = spool.tile([128, USB, L], F16)
                            for k in range(USB):
                                us = USB * b2 + k
                                nc.vector.tensor_scalar_add(
                                    s[:, k, :],
                                    k4_sb[:, us, :],
                                    qb_sb[:, us, ds(g, 1)],
                                )
                            h = hpool.tile([128, USB, L], F16)
                            nc.scalar.activation(h[:], s[:], AF.Tanh)
                            for k in range(USB):
                                us = USB * b2 + k
                                for n in range(L // 512):
                                    nc.tensor.matmul(
                                        e_ps[ds(32 * c, 32), ds(n * 512, 512)],
                                        wa32_sb[:, us, :],
                                        h[:, k, ds(n * 512, 512)],
                                        start=(us == 0),
                                        stop=(us == NUS - 1),
                                        tile_position=(0, 32 * c),
                                    )
                    p = ppool.tile([128, L], F32R)
                    nc.scalar.activation(
                        p[:], e_ps[:], AF.Exp, accum_out=sums_sb[:, ds(qb, 1)]
                    )
                    nc.vector.reciprocal(recip_sb[:, ds(qb, 1)], sums_sb[:, ds(qb, 1)])
                    at_sb = atpool.tile([128, NJC, 128], F32R)
                    at_ps = pe_at.tile([128, L], F32R)
                    for jc in range(NJC):
                        nc.tensor.transpose(
                            at_ps[:, ts(jc, 128)], p[:, ts(jc, 128)], ident_sb[:]
                        )
                    nc.vector.tensor_copy(at_sb[:], at_ps[:])
                    v_ps = pe_v.tile([128, D], F32)
                    for jc in range(NJC):
                        nc.tensor.matmul(
                            v_ps[:],
                            at_sb[:, jc, :],
                            x_sb[:, jc, :],
                            start=(jc == 0),
                            stop=(jc == NJC - 1),
                        )
                    v_sb = vpool.tile([128, D], F32)
                    nc.vector.tensor_scalar_mul(
                        v_sb[:], v_ps[:], recip_sb[:, ds(qb, 1)]
                    )
                    nc.sync.dma_start(out_r[qb], v_sb[:])

    return nc


_NC_CACHE: dict = {}


def get_compiled_nc():
    if "nc" not in _NC_CACHE:
        nc = bacc.Bacc("TRN2", target_bir_lowering=False, debug=False)
        build_kernel(nc)
        nc.compile()
        _NC_CACHE["nc"] = nc
    return _NC_CACHE["nc"]


def make_in_maps(inputs_np, Wt, Wx, bh, Wa):
    wx4 = np.zeros((D, NUS, 128), np.float32)
    wa32 = np.zeros((NUS, 128, GQ), np.float16)
    for us in range(NUS):
        for uu in range(4):
            u = 4 * us + uu
            wx4[:, us, 32 * uu : 32 * (uu + 1)] = Wx[:, u : u + 1]
            wa32[us, 32 * uu : 32 * (uu + 1), :] = Wa[u, 0] * np.eye(GQ, dtype=np.float32)
    bh_c = bh.reshape(U, 1).astype(np.float32)
    ident = np.eye(128, dtype=np.float32)
    in_maps = []
    for c in range(NCORES):
        b, half = divmod(c, HALVES)
        in_maps.append(
            {
                "x": np.ascontiguousarray(inputs_np[b]),
                "xq": np.ascontiguousarray(inputs_np[b, half * LQ : (half + 1) * LQ]),
                "wt": Wt,
                "wx4": wx4,
                "wa32": wa32,
                "bh": bh_c,
                "ident": ident,
            }
        )
    return in_maps


def kernel(**inputs) -> np.ndarray:
    x = np.asarray(inputs["inputs"], dtype=np.float32)
    Wt = np.ascontiguousarray(np.asarray(inputs["Wt"], np.float32))
    Wx = np.ascontiguousarray(np.asarray(inputs["Wx"], np.float32))
    bh = np.asarray(inputs["bh"], np.float32)
    Wa = np.asarray(inputs["Wa"], np.float32)

    from concourse.bass_utils import run_bass_kernel_spmd

    nc = get_compiled_nc()
    in_maps = make_in_maps(x, Wt, Wx, bh, Wa)
    res = run_bass_kernel_spmd(nc, in_maps, list(range(NCORES)))
    kernel._last_results = res  # type: ignore[attr-defined]

    out = np.empty((B, L, D), np.float32)
    for c in range(NCORES):
        b, half = divmod(c, HALVES)
        out[b, half * LQ : (half + 1) * LQ] = res.results[c]["out"]
    return out


# revision 8
# speedup vs baseline: 1.0077x; 1.0031x over previous
"""Bahdanau additive-attention pooling for Trainium2 (Bass/Tile).

Reference math (per batch):
    q = x @ Wt; k = x @ Wx                                  [L, U]
    e[i,j] = sum_u Wa[u] * tanh(q[i,u] + k[j,u] + bh[u])    (+ ba, dropped --
                                                             softmax shift-inv)
    v = softmax_j(e) @ x                                    [L, D]

Sharding: 8 cores = 4 batches x 2 query-halves (data-parallel, no
collectives).  Per core: 512 queries x 1024 keys, flash-style over query
blocks of 128 so the [L, L, U] tensor h is never materialized.

Per-core layout: partitions p = 32*uu + ii, where ii indexes 32 queries of a
"group" and uu 4 of the 32 u's; u-slices us = 0..7 cover u = 4*us+uu.  Groups
are query-strided (group g = queries {16*ii + g}) so every cross-partition
data movement is a clean strided DMA; the output DMA un-permutes.

  K4[us][p, j] = k[j, 4us+uu]      PE matmul, host-replicated Wx4, fp32r
  Qb[us][p, g] = q[16ii+g, ...]+bh qT -> DRAM -> strided gather-back
  S  = K4[us] + Qb[us][:, g]       VectorE tensor_scalar; K4 and S are fp16
                                   (16-bit packed DVE mode, ~2x; halves the
                                   K4 PSUM->SBUF copy payload on ScalarE)
  H  = tanh(S)                     ScalarE, batched 4 u-slices per instr,
                                   fp16 output (the engine bottleneck:
                                   L*L*U/8 = 16.8M lanes-elems per core)
  e[32c:32c+32, :] += wa32[us].T@H PE, M=32 col-tiled at partition base 32c
                                   (fp16: full rate + legal dst partition;
                                   fp32r is full-rate but base-0 only),
                                   8 accumulating matmuls contract u
  P = exp(e)                       ScalarE on the [128q, 1024k] PSUM block,
                                   row-sums via accum_out (|e| <= ~4.5, so
                                   no max-subtraction is needed)
  aT chunks = PE transpose(P); v = sum_jc aT[jc].T @ x[jc] (fp32r); scale by
  1/rowsum on VectorE; DMA out.

Engine budget per core (model): ScalarE ~131us (86% busy - bound by the
16.8M-element tanh volume at 1 elem/cycle/lane @1.2GHz), PE ~76us,
VectorE ~51us, total ~152us.
"""

import numpy as np

import concourse.bass as bass
import concourse.mybir as mybir
import concourse.tile as tile
from concourse import bacc
from concourse.bass import ds, ts

B, L, D, U = 4, 1024, 256, 32
NCORES = 8
HALVES = 2
LQ = L // HALVES                # 512 queries per core
GQ = 32                         # queries per group
NGRP = LQ // GQ                 # 16 groups
NUS = 8                         # u-slices (4 u's each)
USB = 4                         # u-slices per tanh batch
QB = 128                        # query block (softmax granularity)
NQB = LQ // QB                  # 4
NJC = L // 128                  # 8 key chunks
NDC = D // 128                  # 2 contraction chunks

F32 = mybir.dt.float32
F32R = mybir.dt.float32r
F16 = mybir.dt.float16
AF = mybir.ActivationFunctionType


def build_kernel(nc: bass.Bass):
    x_d = nc.dram_tensor("x", [L, D], F32R, kind="ExternalInput")
    xq_d = nc.dram_tensor("xq", [LQ, D], F32R, kind="ExternalInput")
    wt_d = nc.dram_tensor("wt", [D, U], F32R, kind="ExternalInput")
    wx4_d = nc.dram_tensor("wx4", [D, NUS, 128], F32R, kind="ExternalInput")
    wa32_d = nc.dram_tensor("wa32", [NUS, 128, GQ], F16, kind="ExternalInput")
    bh_d = nc.dram_tensor("bh", [U, 1], F32, kind="ExternalInput")
    ident_d = nc.dram_tensor("ident", [128, 128], F32R, kind="ExternalInput")
    out_d = nc.dram_tensor("out", [LQ, D], F32, kind="ExternalOutput")
    qtb_d = nc.dram_tensor("qtb", [U, LQ], F32)  # scratch for the Qb gather

    with tile.TileContext(nc) as tc:
        with tc.tile_pool(name="const", bufs=1) as cpool:
            x_sb = cpool.tile([128, NJC, D], F32R)
            xq_sb = cpool.tile([128, NQB, D], F32R)
            xT_sb = cpool.tile([128, NDC, L], F32R)
            xqT_sb = cpool.tile([128, NDC, LQ], F32R)
            wt_sb = cpool.tile([128, NDC, U], F32R)
            wx4_sb = cpool.tile([128, NDC, NUS, 128], F32R)
            wa32_sb = cpool.tile([128, NUS, GQ], F16)
            bh_sb = cpool.tile([U, 1], F32)
            ident_sb = cpool.tile([128, 128], F32R)
            k4_sb = cpool.tile([128, NUS, L], F16)
            qtb_sb = cpool.tile([U, LQ], F32)
            qb_sb = cpool.tile([128, NUS, NGRP], F32)
            sums_sb = cpool.tile([128, NQB], F32)
            recip_sb = cpool.tile([128, NQB], F32)

            # small/critical DMAs first; 1MB wx4 split per-us and last
            nc.sync.dma_start(ident_sb[:], ident_d.ap())
            nc.scalar.dma_start(bh_sb[:], bh_d.ap())
            nc.scalar.dma_start(
                wt_sb[:], wt_d.ap().rearrange("(c p) u -> p c u", p=128)
            )
            nc.scalar.dma_start(
                wa32_sb[:], wa32_d.ap().rearrange("us p m -> p us m")
            )
            xq_r = xq_d.ap().rearrange("(c p) d -> c p d", p=128)
            for jc in range(NQB):
                nc.sync.dma_start(xq_sb[:, jc, :], xq_r[jc])
            x_r = x_d.ap().rearrange("(c p) d -> c p d", p=128)
            wx4_r = wx4_d.ap().rearrange("(c p) us m -> p c us m", p=128)
            for jc in (0, 2):
                nc.sync.dma_start(x_sb[:, jc, :], x_r[jc])
            for jc in (1, 3):
                nc.gpsimd.dma_start(x_sb[:, jc, :], x_r[jc])
            # first wx4 slices early: they gate the first K4 matmuls
            for us in (0, 1):
                nc.gpsimd.dma_start(wx4_sb[:, :, us, :], wx4_r[:, :, us, :])
            for jc in (4, 6):
                nc.sync.dma_start(x_sb[:, jc, :], x_r[jc])
            for jc in (5, 7):
                nc.gpsimd.dma_start(x_sb[:, jc, :], x_r[jc])
            for us in range(2, NUS):
                nc.gpsimd.dma_start(wx4_sb[:, :, us, :], wx4_r[:, :, us, :])

            # ---- prologue ----
            with (
                tc.tile_pool(name="ptr", bufs=3, space="PSUM") as ptr,
                tc.tile_pool(name="pk4", bufs=2, space="PSUM") as pk4,
                tc.tile_pool(name="pqt", bufs=1, space="PSUM") as pqt,
            ):
                # xq^T first: the qT -> DRAM -> gather chain is the longest
                for dc in range(NDC):
                    tr4 = ptr.tile([128, 512], F32R)
                    for jc in range(NQB):
                        nc.tensor.transpose(
                            tr4[:, ts(jc, 128)],
                            xq_sb[:, jc, ds(dc * 128, 128)],
                            ident_sb[:],
                        )
                    nc.scalar.copy(xqT_sb[:, dc, :], tr4[:])
                qt_ps = pqt.tile([U, LQ], F32)
                for dc in range(NDC):
                    nc.tensor.matmul(
                        qt_ps[:],
                        wt_sb[:, dc, :],
                        xqT_sb[:, dc, :],
                        start=(dc == 0),
                        stop=(dc == NDC - 1),
                    )
                nc.vector.tensor_scalar_add(qtb_sb[:], qt_ps[:], bh_sb[:])
                nc.sync.dma_start(qtb_d.ap(), qtb_sb[:])
                # Qb[us][32uu+ii, g] = qtb[4us+uu, 16ii+g]  (strided groups:
                # group g holds queries {16ii+g}) -> contiguous 64B runs
                qtb_r = qtb_d.ap().rearrange(
                    "(us uu) (ii g) -> uu ii us g", uu=4, g=NGRP
                )
                for uu in range(4):
                    dst = qb_sb[ds(32 * uu, GQ), :, :]
                    nc.sync.dma_start(dst, qtb_r[uu])

                # x^T: 4 chunk-transposes per PSUM tile, one copy per tile
                for n in range(L // 512):
                    for dc in range(NDC):
                        tr4 = ptr.tile([128, 512], F32R)
                        for q4 in range(4):
                            jc = 4 * n + q4
                            nc.tensor.transpose(
                                tr4[:, ts(q4, 128)],
                                x_sb[:, jc, ds(dc * 128, 128)],
                                ident_sb[:],
                            )
                        if dc == 0:
                            nc.vector.tensor_copy(
                                xT_sb[:, dc, ds(n * 512, 512)], tr4[:]
                            )
                        else:
                            nc.scalar.copy(
                                xT_sb[:, dc, ds(n * 512, 512)], tr4[:]
                            )

                # K4[us] = k^T slice-replicated, via host-replicated Wx4
                for us in range(NUS):
                    kp = pk4.tile([128, L], F32)
                    for n in range(L // 512):
                        for dc in range(NDC):
                            nc.tensor.matmul(
                                kp[:, ds(n * 512, 512)],
                                wx4_sb[:, dc, us, :],
                                xT_sb[:, dc, ds(n * 512, 512)],
                                start=(dc == 0),
                                stop=(dc == NDC - 1),
                            )
                    nc.scalar.copy(k4_sb[:, us, :], kp[:])

            # ---- main loop ----
            with (
                tc.tile_pool(name="spool", bufs=3) as spool,
                tc.tile_pool(name="hpool", bufs=3) as hpool,
                tc.tile_pool(name="ppool", bufs=2) as ppool,
                tc.tile_pool(name="atpool", bufs=2) as atpool,
                tc.tile_pool(name="vpool", bufs=2) as vpool,
                tc.tile_pool(name="pe", bufs=2, space="PSUM") as pe_e,
                tc.tile_pool(name="pat", bufs=1, space="PSUM") as pe_at,
                tc.tile_pool(name="pv", bufs=2, space="PSUM") as pe_v,
            ):
                out_r = out_d.ap().rearrange(
                    "(ii gg c) d -> gg c ii d", gg=NQB, c=4
                )
                for qb in range(NQB):
                    e_ps = pe_e.tile([128, L], F32)
                    for c in range(4):
                        g = 4 * qb + c
                        for b2 in range(NUS // USB):
                            s = spool.tile([128, USB, L], F16)
                            for k in range(USB):
                                us = USB * b2 + k
                                nc.vector.tensor_scalar_add(
                                    s[:, k, :],
                                    k4_sb[:, us, :],
                                    qb_sb[:, us, ds(g, 1)],
                                )
                            h = hpool.tile([128, USB, L], F16)
                            nc.scalar.activation(h[:], s[:], AF.Tanh)
                            for k in range(USB):
                                us = USB * b2 + k
                                for n in range(L // 512):
                                    nc.tensor.matmul(
                                        e_ps[ds(32 * c, 32), ds(n * 512, 512)],
                                        wa32_sb[:, us, :],
                                        h[:, k, ds(n * 512, 512)],
                                        start=(us == 0),
                                        stop=(us == NUS - 1),
                                        tile_position=(0, 32 * c),
                                    )
                    p = ppool.tile([128, L], F32R)
                    nc.scalar.activation(
                        p[:], e_ps[:], AF.Exp, accum_out=sums_sb[:, ds(qb, 1)]
                    )
                    nc.vector.reciprocal(recip_sb[:, ds(qb, 1)], sums_sb[:, ds(qb, 1)])
                    at_sb = atpool.tile([128, NJC, 128], F32R)
                    at_ps = pe_at.tile([128, L], F32R)
                    for jc in range(NJC):
                        nc.tensor.transpose(
                            at_ps[:, ts(jc, 128)], p[:, ts(jc, 128)], ident_sb[:]
                        )
                    nc.vector.tensor_copy(at_sb[:], at_ps[:])
                    v_ps = pe_v.tile([128, D], F32)
                    for jc in range(NJC):
                        nc.tensor.matmul(
                            v_ps[:],
                            at_sb[:, jc, :],
                            x_sb[:, jc, :],
                            start=(jc == 0),
                            stop=(jc == NJC - 1),
                        )
                    v_sb = vpool.tile([128, D], F32)
                    nc.vector.tensor_scalar_mul(
                        v_sb[:], v_ps[:], recip_sb[:, ds(qb, 1)]
                    )
                    nc.sync.dma_start(out_r[qb], v_sb[:])

    return nc


_NC_CACHE: dict = {}


def get_compiled_nc():
    if "nc" not in _NC_CACHE:
        nc = bacc.Bacc("TRN2", target_bir_lowering=False, debug=False)
        build_kernel(nc)
        nc.compile()
        _NC_CACHE["nc"] = nc
    return _NC_CACHE["nc"]


def make_in_maps(inputs_np, Wt, Wx, bh, Wa):
    wx4 = np.zeros((D, NUS, 128), np.float32)
    wa32 = np.zeros((NUS, 128, GQ), np.float16)
    for us in range(NUS):
        for uu in range(4):
            u = 4 * us + uu
            wx4[:, us, 32 * uu : 32 * (uu + 1)] = Wx[:, u : u + 1]
            wa32[us, 32 * uu : 32 * (uu + 1), :] = Wa[u, 0] * np.eye(GQ, dtype=np.float32)
    bh_c = bh.reshape(U, 1).astype(np.float32)
    ident = np.eye(128, dtype=np.float32)
    in_maps = []
    for c in range(NCORES):
        b, half = divmod(c, HALVES)
        in_maps.append(
            {
                "x": np.ascontiguousarray(inputs_np[b]),
                "xq": np.ascontiguousarray(inputs_np[b, half * LQ : (half + 1) * LQ]),
                "wt": Wt,
                "wx4": wx4,
                "wa32": wa32,
                "bh": bh_c,
                "ident": ident,
            }
        )
    return in_maps


def kernel(**inputs) -> np.ndarray:
    x = np.asarray(inputs["inputs"], dtype=np.float32)
    Wt = np.ascontiguousarray(np.asarray(inputs["Wt"], np.float32))
    Wx = np.ascontiguousarray(np.asarray(inputs["Wx"], np.float32))
    bh = np.asarray(inputs["bh"], np.float32)
    Wa = np.asarray(inputs["Wa"], np.float32)

    from concourse.bass_utils import run_bass_kernel_spmd

    nc = get_compiled_nc()
    in_maps = make_in_maps(x, Wt, Wx, bh, Wa)
    res = run_bass_kernel_spmd(nc, in_maps, list(range(NCORES)))
    kernel._last_results = res  # type: ignore[attr-defined]

    out = np.empty((B, L, D), np.float32)
    for c in range(NCORES):
        b, half = divmod(c, HALVES)
        out[b, half * LQ : (half + 1) * LQ] = res.results[c]["out"]
    return out


# revision 9
# speedup vs baseline: 1.0175x; 1.0097x over previous
"""Bahdanau additive-attention pooling for Trainium2 (Bass/Tile).

Reference math (per batch):
    q = x @ Wt; k = x @ Wx                                  [L, U]
    e[i,j] = sum_u Wa[u] * tanh(q[i,u] + k[j,u] + bh[u])    (+ ba, dropped --
                                                             softmax shift-inv)
    v = softmax_j(e) @ x                                    [L, D]

Sharding: 8 cores = 4 batches x 2 query-halves (data-parallel, no
collectives).  Per core: 512 queries x 1024 keys, flash-style over query
blocks of 128 so the [L, L, U] tensor h is never materialized.

Per-core layout: partitions p = 32*uu + ii, where ii indexes 32 queries of a
"group" and uu 4 of the 32 u's; u-slices us = 0..7 cover u = 4*us+uu.  Groups
are query-strided (group g = queries {16*ii + g}) so every cross-partition
data movement is a clean strided DMA; the output DMA un-permutes.

  K4[us][p, j] = k[j, 4us+uu]      PE matmul, host-replicated Wx4, fp32r
  Qb[us][p, g] = q[16ii+g, ...]+bh qT -> DRAM -> strided gather-back
  S  = K4[us] + Qb[us][:, g]       VectorE tensor_scalar; K4 and S are fp16
                                   (16-bit packed DVE mode, ~2x; halves the
                                   K4 PSUM->SBUF copy payload on ScalarE)
  H  = tanh(S)                     ScalarE, batched 4 u-slices per instr,
                                   fp16 output (the engine bottleneck:
                                   L*L*U/8 = 16.8M lanes-elems per core)
  e[32c:32c+32, :] += wa32[us].T@H PE, M=32 col-tiled at partition base 32c
                                   (fp16: full rate + legal dst partition;
                                   fp32r is full-rate but base-0 only),
                                   8 accumulating matmuls contract u
  P = exp(e)                       ScalarE on the [128q, 1024k] PSUM block,
                                   row-sums via accum_out (|e| <= ~4.5, so
                                   no max-subtraction is needed)
  aT chunks = PE transpose(P); v = sum_jc aT[jc].T @ x[jc] (fp32r); scale by
  1/rowsum on VectorE; DMA out.

Engine budget per core (model): ScalarE ~131us (86% busy - bound by the
16.8M-element tanh volume at 1 elem/cycle/lane @1.2GHz), PE ~76us,
VectorE ~51us, total ~152us.
"""

import numpy as np

import concourse.bass as bass
import concourse.mybir as mybir
import concourse.tile as tile
from concourse import bacc
from concourse.bass import ds, ts

B, L, D, U = 4, 1024, 256, 32
NCORES = 8
HALVES = 2
LQ = L // HALVES                # 512 queries per core
GQ = 32                         # queries per group
NGRP = LQ // GQ                 # 16 groups
NUS = 8                         # u-slices (4 u's each)
USB = 4                         # u-slices per tanh batch
QB = 128                        # query block (softmax granularity)
NQB = LQ // QB                  # 4
NJC = L // 128                  # 8 key chunks
NDC = D // 128                  # 2 contraction chunks

F32 = mybir.dt.float32
F32R = mybir.dt.float32r
F16 = mybir.dt.float16
AF = mybir.ActivationFunctionType


def build_kernel(nc: bass.Bass):
    x_d = nc.dram_tensor("x", [L, D], F32R, kind="ExternalInput")
    xq_d = nc.dram_tensor("xq", [LQ, D], F32R, kind="ExternalInput")
    wt_d = nc.dram_tensor("wt", [D, U], F32R, kind="ExternalInput")
    wx4_d = nc.dram_tensor("wx4", [D, NUS, 128], F32R, kind="ExternalInput")
    wa32_d = nc.dram_tensor("wa32", [NUS, 128, GQ], F16, kind="ExternalInput")
    bh_d = nc.dram_tensor("bh", [U, 1], F32, kind="ExternalInput")
    ident_d = nc.dram_tensor("ident", [128, 128], F32R, kind="ExternalInput")
    out_d = nc.dram_tensor("out", [LQ, D], F32, kind="ExternalOutput")
    qtb_d = nc.dram_tensor("qtb", [U, LQ], F32)  # scratch for the Qb gather

    with tile.TileContext(nc) as tc:
        with tc.tile_pool(name="const", bufs=1) as cpool:
            x_sb = cpool.tile([128, NJC, D], F32R)
            xq_sb = cpool.tile([128, NQB, D], F32R)
            xT_sb = cpool.tile([128, NDC, L], F32R)
            xqT_sb = cpool.tile([128, NDC, LQ], F32R)
            wt_sb = cpool.tile([128, NDC, U], F32R)
            wx4_sb = cpool.tile([128, NDC, NUS, 128], F32R)
            wa32_sb = cpool.tile([128, NUS, GQ], F16)
            bh_sb = cpool.tile([U, 1], F32)
            ident_sb = cpool.tile([128, 128], F32R)
            k4_sb = cpool.tile([128, NUS, L], F16)
            qtb_sb = cpool.tile([U, LQ], F32)
            qb_sb = cpool.tile([128, NUS, NGRP], F32)
            sums_sb = cpool.tile([128, NQB], F32)
            recip_sb = cpool.tile([128, NQB], F32)

            # small/critical DMAs first; 1MB wx4 split per-us and last
            nc.sync.dma_start(ident_sb[:], ident_d.ap())
            nc.scalar.dma_start(bh_sb[:], bh_d.ap())
            nc.scalar.dma_start(
                wt_sb[:], wt_d.ap().rearrange("(c p) u -> p c u", p=128)
            )
            nc.scalar.dma_start(
                wa32_sb[:], wa32_d.ap().rearrange("us p m -> p us m")
            )
            nc.sync.dma_start(
                xq_sb[:], xq_d.ap().rearrange("(c p) d -> p c d", p=128)
            )
            x_r = x_d.ap().rearrange("(c p) d -> c p d", p=128)
            wx4_r = wx4_d.ap().rearrange("(c p) us m -> p c us m", p=128)
            for jc in (0, 2):
                nc.sync.dma_start(x_sb[:, jc, :], x_r[jc])
            for jc in (1, 3):
                nc.gpsimd.dma_start(x_sb[:, jc, :], x_r[jc])
            # first wx4 slices early: they gate the first K4 matmuls
            for us in (0, 1):
                nc.gpsimd.dma_start(wx4_sb[:, :, us, :], wx4_r[:, :, us, :])
            for jc in (4, 6):
                nc.sync.dma_start(x_sb[:, jc, :], x_r[jc])
            for jc in (5, 7):
                nc.gpsimd.dma_start(x_sb[:, jc, :], x_r[jc])
            for us in range(2, NUS):
                nc.gpsimd.dma_start(wx4_sb[:, :, us, :], wx4_r[:, :, us, :])

            # ---- prologue ----
            with (
                tc.tile_pool(name="ptr", bufs=3, space="PSUM") as ptr,
                tc.tile_pool(name="pk4", bufs=2, space="PSUM") as pk4,
                tc.tile_pool(name="pqt", bufs=1, space="PSUM") as pqt,
            ):
                # xq^T first: the qT -> DRAM -> gather chain is the longest
                for dc in range(NDC):
                    tr4 = ptr.tile([128, 512], F32R)
                    for jc in range(NQB):
                        nc.tensor.transpose(
                            tr4[:, ts(jc, 128)],
                            xq_sb[:, jc, ds(dc * 128, 128)],
                            ident_sb[:],
                        )
                    nc.scalar.copy(xqT_sb[:, dc, :], tr4[:])
                qt_ps = pqt.tile([U, LQ], F32)
                for dc in range(NDC):
                    nc.tensor.matmul(
                        qt_ps[:],
                        wt_sb[:, dc, :],
                        xqT_sb[:, dc, :],
                        start=(dc == 0),
                        stop=(dc == NDC - 1),
                    )
                nc.vector.tensor_scalar_add(qtb_sb[:], qt_ps[:], bh_sb[:])
                nc.sync.dma_start(qtb_d.ap(), qtb_sb[:])
                # Qb[us][32uu+ii, g] = qtb[4us+uu, 16ii+g]  (strided groups:
                # group g holds queries {16ii+g}) -> contiguous 64B runs
                qtb_r = qtb_d.ap().rearrange(
                    "(us uu) (ii g) -> uu ii us g", uu=4, g=NGRP
                )
                for uu in range(4):
                    dst = qb_sb[ds(32 * uu, GQ), :, :]
                    nc.sync.dma_start(dst, qtb_r[uu])

                # x^T: 4 chunk-transposes per PSUM tile, one copy per tile
                for n in range(L // 512):
                    for dc in range(NDC):
                        tr4 = ptr.tile([128, 512], F32R)
                        for q4 in range(4):
                            jc = 4 * n + q4
                            nc.tensor.transpose(
                                tr4[:, ts(q4, 128)],
                                x_sb[:, jc, ds(dc * 128, 128)],
                                ident_sb[:],
                            )
                        if dc == 0:
                            nc.vector.tensor_copy(
                                xT_sb[:, dc, ds(n * 512, 512)], tr4[:]
                            )
                        else:
                            nc.scalar.copy(
                                xT_sb[:, dc, ds(n * 512, 512)], tr4[:]
                            )

                # K4[us] = k^T slice-replicated, via host-replicated Wx4
                for us in range(NUS):
                    kp = pk4.tile([128, L], F32)
                    for n in range(L // 512):
                        for dc in range(NDC):
                            nc.tensor.matmul(
                                kp[:, ds(n * 512, 512)],
                                wx4_sb[:, dc, us, :],
                                xT_sb[:, dc, ds(n * 512, 512)],
                                start=(dc == 0),
                                stop=(dc == NDC - 1),
                            )
                    nc.scalar.copy(k4_sb[:, us, :], kp[:])

            # ---- main loop ----
            with (
                tc.tile_pool(name="spool", bufs=3) as spool,
                tc.tile_pool(name="hpool", bufs=3) as hpool,
                tc.tile_pool(name="ppool", bufs=2) as ppool,
                tc.tile_pool(name="atpool", bufs=2) as atpool,
                tc.tile_pool(name="vpool", bufs=2) as vpool,
                tc.tile_pool(name="pe", bufs=2, space="PSUM") as pe_e,
                tc.tile_pool(name="pat", bufs=1, space="PSUM") as pe_at,
                tc.tile_pool(name="pv", bufs=2, space="PSUM") as pe_v,
            ):
                out_r = out_d.ap().rearrange(
                    "(ii gg c) d -> gg c ii d", gg=NQB, c=4
                )
                for qb in range(NQB):
                    e_ps = pe_e.tile([128, L], F32)
                    for c in range(4):
                        g = 4 * qb + c
                        # the very last group's final batch is split 2+2 so
                        # the e-matmul stretch after the last tanh (which
                        # gates the final exp) is half as long
                        last = qb == NQB - 1 and c == 3
                        batches = [(0, 4), (4, 2), (6, 2)] if last else [
                            (0, USB), (USB, USB)
                        ]
                        for us0, usn in batches:
                            s = spool.tile([128, USB, L], F16, tag="s")
                            for k in range(usn):
                                us = us0 + k
                                nc.vector.tensor_scalar_add(
                                    s[:, k, :],
                                    k4_sb[:, us, :],
                                    qb_sb[:, us, ds(g, 1)],
                                )
                            h = hpool.tile([128, USB, L], F16, tag="h")
                            nc.scalar.activation(
                                h[:, 0:usn, :], s[:, 0:usn, :], AF.Tanh
                            )
                            for k in range(usn):
                                us = us0 + k
                                for n in range(L // 512):
                                    nc.tensor.matmul(
                                        e_ps[ds(32 * c, 32), ds(n * 512, 512)],
                                        wa32_sb[:, us, :],
                                        h[:, k, ds(n * 512, 512)],
                                        start=(us == 0),
                                        stop=(us == NUS - 1),
                                        tile_position=(0, 32 * c),
                                    )
                    p = ppool.tile([128, L], F32R)
                    nc.scalar.activation(
                        p[:], e_ps[:], AF.Exp, accum_out=sums_sb[:, ds(qb, 1)]
                    )
                    nc.vector.reciprocal(recip_sb[:, ds(qb, 1)], sums_sb[:, ds(qb, 1)])
                    at_sb = atpool.tile([128, NJC, 128], F32R)
                    at_ps = pe_at.tile([128, L], F32R)
                    for jc in range(NJC):
                        nc.tensor.transpose(
                            at_ps[:, ts(jc, 128)], p[:, ts(jc, 128)], ident_sb[:]
                        )
                    if qb == NQB - 1:
                        # ACT is done after the last exp; split the copy
                        nc.vector.tensor_copy(
                            at_sb[:, 0 : NJC // 2, :], at_ps[:, 0 : L // 2]
                        )
                        nc.scalar.copy(
                            at_sb[:, NJC // 2 :, :], at_ps[:, L // 2 :]
                        )
                    else:
                        nc.vector.tensor_copy(at_sb[:], at_ps[:])
                    v_ps = pe_v.tile([128, D], F32)
                    for jc in range(NJC):
                        nc.tensor.matmul(
                            v_ps[:],
                            at_sb[:, jc, :],
                            x_sb[:, jc, :],
                            start=(jc == 0),
                            stop=(jc == NJC - 1),
                        )
                    v_sb = vpool.tile([128, D], F32)
                    nc.vector.tensor_scalar_mul(
                        v_sb[:], v_ps[:], recip_sb[:, ds(qb, 1)]
                    )
                    nc.sync.dma_start(out_r[qb], v_sb[:])

    return nc


_NC_CACHE: dict = {}


def get_compiled_nc():
    if "nc" not in _NC_CACHE:
        nc = bacc.Bacc("TRN2", target_bir_lowering=False, debug=False)
        build_kernel(nc)
        nc.compile()
        _NC_CACHE["nc"] = nc
    return _NC_CACHE["nc"]


def make_in_maps(inputs_np, Wt, Wx, bh, Wa):
    wx4 = np.zeros((D, NUS, 128), np.float32)
    wa32 = np.zeros((NUS, 128, GQ), np.float16)
    for us in range(NUS):
        for uu in range(4):
            u = 4 * us + uu
            wx4[:, us, 32 * uu : 32 * (uu + 1)] = Wx[:, u : u + 1]
            wa32[us, 32 * uu : 32 * (uu + 1), :] = Wa[u, 0] * np.eye(GQ, dtype=np.float32)
    bh_c = bh.reshape(U, 1).astype(np.float32)
    ident = np.eye(128, dtype=np.float32)
    in_maps = []
    for c in range(NCORES):
        b, half = divmod(c, HALVES)
        in_maps.append(
            {
                "x": np.ascontiguousarray(inputs_np[b]),
                "xq": np.ascontiguousarray(inputs_np[b, half * LQ : (half + 1) * LQ]),
                "wt": Wt,
                "wx4": wx4,
                "wa32": wa32,
                "bh": bh_c,
                "ident": ident,
            }
        )
    return in_maps


def kernel(**inputs) -> np.ndarray:
    x = np.asarray(inputs["inputs"], dtype=np.float32)
    Wt = np.ascontiguousarray(np.asarray(inputs["Wt"], np.float32))
    Wx = np.ascontiguousarray(np.asarray(inputs["Wx"], np.float32))
    bh = np.asarray(inputs["bh"], np.float32)
    Wa = np.asarray(inputs["Wa"], np.float32)

    from concourse.bass_utils import run_bass_kernel_spmd

    nc = get_compiled_nc()
    in_maps = make_in_maps(x, Wt, Wx, bh, Wa)
    res = run_bass_kernel_spmd(nc, in_maps, list(range(NCORES)))
    kernel._last_results = res  # type: ignore[attr-defined]

    out = np.empty((B, L, D), np.float32)
    for c in range(NCORES):
        b, half = divmod(c, HALVES)
        out[b, half * LQ : (half + 1) * LQ] = res.results[c]["out"]
    return out


# revision 10
# speedup vs baseline: 1.0261x; 1.0085x over previous
"""Bahdanau additive-attention pooling for Trainium2 (Bass/Tile).

Reference math (per batch):
    q = x @ Wt; k = x @ Wx                                  [L, U]
    e[i,j] = sum_u Wa[u] * tanh(q[i,u] + k[j,u] + bh[u])    (+ ba, dropped --
                                                             softmax shift-inv)
    v = softmax_j(e) @ x                                    [L, D]

Sharding: 8 cores = 4 batches x 2 query-halves (data-parallel, no
collectives).  Per core: 512 queries x 1024 keys, flash-style over query
blocks of 128 so the [L, L, U] tensor h is never materialized.

Per-core layout: partitions p = 32*uu + ii, where ii indexes 32 queries of a
"group" and uu 4 of the 32 u's; u-slices us = 0..7 cover u = 4*us+uu.  Groups
are query-strided (group g = queries {16*ii + g}) so every cross-partition
data movement is a clean strided DMA; the output DMA un-permutes.

  K4[us][p, j] = k[j, 4us+uu]      PE matmul, host-replicated Wx4, fp32r
  Qb[us][p, g] = q[16ii+g, ...]+bh qT -> DRAM -> strided gather-back
  S  = K4[us] + Qb[us][:, g]       VectorE tensor_scalar; K4 and S are fp16
                                   (16-bit packed DVE mode, ~2x; halves the
                                   K4 PSUM->SBUF copy payload on ScalarE)
  H  = tanh(S)                     ScalarE, batched 4 u-slices per instr,
                                   fp16 output (the engine bottleneck:
                                   L*L*U/8 = 16.8M lanes-elems per core)
  e[32c:32c+32, :] += wa32[us].T@H PE, M=32 col-tiled at partition base 32c
                                   (fp16: full rate + legal dst partition;
                                   fp32r is full-rate but base-0 only),
                                   8 accumulating matmuls contract u
  P = exp(e)                       ScalarE on the [128q, 1024k] PSUM block,
                                   row-sums via accum_out (|e| <= ~4.5, so
                                   no max-subtraction is needed)
  aT chunks = PE transpose(P); v = sum_jc aT[jc].T @ x[jc] (fp32r); scale by
  1/rowsum on VectorE; DMA out.

Engine budget per core (model): ScalarE ~131us (86% busy - bound by the
16.8M-element tanh volume at 1 elem/cycle/lane @1.2GHz), PE ~76us,
VectorE ~51us, total ~152us.
"""

import numpy as np

import concourse.bass as bass
import concourse.mybir as mybir
import concourse.tile as tile
from concourse import bacc
from concourse.bass import ds, ts

B, L, D, U = 4, 1024, 256, 32
NCORES = 8
HALVES = 2
LQ = L // HALVES                # 512 queries per core
GQ = 32                         # queries per group
NGRP = LQ // GQ                 # 16 groups
NUS = 8                         # u-slices (4 u's each)
USB = 4                         # u-slices per tanh batch
QB = 128                        # query block (softmax granularity)
NQB = LQ // QB                  # 4
NJC = L // 128                  # 8 key chunks
NDC = D // 128                  # 2 contraction chunks

F32 = mybir.dt.float32
F32R = mybir.dt.float32r
F16 = mybir.dt.float16
AF = mybir.ActivationFunctionType


def build_kernel(nc: bass.Bass):
    x_d = nc.dram_tensor("x", [L, D], F32R, kind="ExternalInput")
    xq_d = nc.dram_tensor("xq", [LQ, D], F32R, kind="ExternalInput")
    wt_d = nc.dram_tensor("wt", [D, U], F32R, kind="ExternalInput")
    wx4_d = nc.dram_tensor("wx4", [D, NUS, 128], F32R, kind="ExternalInput")
    wa32_d = nc.dram_tensor("wa32", [NUS, 128, GQ], F16, kind="ExternalInput")
    bh_d = nc.dram_tensor("bh", [U, 1], F32, kind="ExternalInput")
    ident_d = nc.dram_tensor("ident", [128, 128], F32R, kind="ExternalInput")
    out_d = nc.dram_tensor("out", [LQ, D], F32, kind="ExternalOutput")
    qtb_d = nc.dram_tensor("qtb", [U, LQ], F32)  # scratch for the Qb gather

    with tile.TileContext(nc) as tc:
        with tc.tile_pool(name="const", bufs=1) as cpool:
            x_sb = cpool.tile([128, NJC, D], F32R)
            xq_sb = cpool.tile([128, NQB, D], F32R)
            xT_sb = cpool.tile([128, NDC, L], F32R)
            xqT_sb = cpool.tile([128, NDC, LQ], F32R)
            wt_sb = cpool.tile([128, NDC, U], F32R)
            wx4_sb = cpool.tile([128, NDC, NUS, 128], F32R)
            wa32_sb = cpool.tile([128, NUS, GQ], F16)
            bh_sb = cpool.tile([U, 1], F32)
            ident_sb = cpool.tile([128, 128], F32R)
            k4_sb = cpool.tile([128, NUS, L], F16)
            qtb_sb = cpool.tile([U, LQ], F32)
            qb_sb = cpool.tile([128, NUS, NGRP], F32)
            sums_sb = cpool.tile([128, NQB], F32)
            recip_sb = cpool.tile([128, NQB], F32)

            # small/critical DMAs first; 1MB wx4 split per-us and last
            nc.scalar.dma_start(ident_sb[:], ident_d.ap())
            nc.scalar.dma_start(bh_sb[:], bh_d.ap())
            nc.scalar.dma_start(
                wt_sb[:], wt_d.ap().rearrange("(c p) u -> p c u", p=128)
            )
            nc.scalar.dma_start(
                wa32_sb[:], wa32_d.ap().rearrange("us p m -> p us m")
            )
            nc.sync.dma_start(
                xq_sb[:], xq_d.ap().rearrange("(c p) d -> p c d", p=128)
            )
            x_r = x_d.ap().rearrange("(c p) d -> c p d", p=128)
            wx4_r = wx4_d.ap().rearrange("(c p) us m -> p c us m", p=128)
            for jc in (0, 2):
                nc.sync.dma_start(x_sb[:, jc, :], x_r[jc])
            for jc in (1, 3):
                nc.gpsimd.dma_start(x_sb[:, jc, :], x_r[jc])
            # first wx4 slices early: they gate the first K4 matmuls
            for us in (0, 1):
                nc.gpsimd.dma_start(wx4_sb[:, :, us, :], wx4_r[:, :, us, :])
            for jc in (5, 7):
                nc.gpsimd.dma_start(x_sb[:, jc, :], x_r[jc])
            for us in range(2, NUS):
                nc.gpsimd.dma_start(wx4_sb[:, :, us, :], wx4_r[:, :, us, :])

            # ---- prologue ----
            with (
                tc.tile_pool(name="ptr", bufs=3, space="PSUM") as ptr,
                tc.tile_pool(name="pk4", bufs=2, space="PSUM") as pk4,
                tc.tile_pool(name="pqt", bufs=1, space="PSUM") as pqt,
            ):
                # xq^T first: the qT -> DRAM -> gather chain is the longest
                for dc in range(NDC):
                    tr4 = ptr.tile([128, 512], F32R)
                    for jc in range(NQB):
                        nc.tensor.transpose(
                            tr4[:, ts(jc, 128)],
                            xq_sb[:, jc, ds(dc * 128, 128)],
                            ident_sb[:],
                        )
                    nc.scalar.copy(xqT_sb[:, dc, :], tr4[:])
                qt_ps = pqt.tile([U, LQ], F32)
                for dc in range(NDC):
                    nc.tensor.matmul(
                        qt_ps[:],
                        wt_sb[:, dc, :],
                        xqT_sb[:, dc, :],
                        start=(dc == 0),
                        stop=(dc == NDC - 1),
                    )
                nc.vector.tensor_scalar_add(qtb_sb[:], qt_ps[:], bh_sb[:])
                nc.sync.dma_start(qtb_d.ap(), qtb_sb[:])
                # Qb[us][32uu+ii, g] = qtb[4us+uu, 16ii+g]  (strided groups:
                # group g holds queries {16ii+g}) -> contiguous 64B runs
                qtb_r = qtb_d.ap().rearrange(
                    "(us uu) (ii g) -> uu ii us g", uu=4, g=NGRP
                )
                for uu in range(4):
                    dst = qb_sb[ds(32 * uu, GQ), :, :]
                    nc.sync.dma_start(dst, qtb_r[uu])
                # x4/x6 queued after the Qb gathers: not needed until the
                # second transpose wave, and ahead of them they delay Qb
                for jc in (4, 6):
                    nc.sync.dma_start(x_sb[:, jc, :], x_r[jc])

                # x^T: 4 chunk-transposes per PSUM tile, one copy per tile
                for n in range(L // 512):
                    for dc in range(NDC):
                        tr4 = ptr.tile([128, 512], F32R)
                        for q4 in range(4):
                            jc = 4 * n + q4
                            nc.tensor.transpose(
                                tr4[:, ts(q4, 128)],
                                x_sb[:, jc, ds(dc * 128, 128)],
                                ident_sb[:],
                            )
                        if dc == 0:
                            nc.vector.tensor_copy(
                                xT_sb[:, dc, ds(n * 512, 512)], tr4[:]
                            )
                        else:
                            nc.scalar.copy(
                                xT_sb[:, dc, ds(n * 512, 512)], tr4[:]
                            )

                # K4[us] = k^T slice-replicated, via host-replicated Wx4
                for us in range(NUS):
                    kp = pk4.tile([128, L], F32)
                    for n in range(L // 512):
                        for dc in range(NDC):
                            nc.tensor.matmul(
                                kp[:, ds(n * 512, 512)],
                                wx4_sb[:, dc, us, :],
                                xT_sb[:, dc, ds(n * 512, 512)],
                                start=(dc == 0),
                                stop=(dc == NDC - 1),
                            )
                    nc.scalar.copy(k4_sb[:, us, :], kp[:])

            # ---- main loop ----
            with (
                tc.tile_pool(name="spool", bufs=3) as spool,
                tc.tile_pool(name="hpool", bufs=3) as hpool,
                tc.tile_pool(name="ppool", bufs=2) as ppool,
                tc.tile_pool(name="atpool", bufs=2) as atpool,
                tc.tile_pool(name="vpool", bufs=2) as vpool,
                tc.tile_pool(name="pe", bufs=2, space="PSUM") as pe_e,
                tc.tile_pool(name="pat", bufs=1, space="PSUM") as pe_at,
                tc.tile_pool(name="pv", bufs=2, space="PSUM") as pe_v,
            ):
                out_r = out_d.ap().rearrange(
                    "(ii gg c) d -> gg c ii d", gg=NQB, c=4
                )
                for qb in range(NQB):
                    e_ps = pe_e.tile([128, L], F32)
                    for c in range(4):
                        g = 4 * qb + c
                        # the very last group's final batch is split 2+2 so
                        # the e-matmul stretch after the last tanh (which
                        # gates the final exp) is half as long
                        last = qb == NQB - 1 and c == 3
                        batches = [(0, 4), (4, 2), (6, 2)] if last else [
                            (0, USB), (USB, USB)
                        ]
                        for us0, usn in batches:
                            s = spool.tile([128, USB, L], F16, tag="s")
                            for k in range(usn):
                                us = us0 + k
                                nc.vector.tensor_scalar_add(
                                    s[:, k, :],
                                    k4_sb[:, us, :],
                                    qb_sb[:, us, ds(g, 1)],
                                )
                            h = hpool.tile([128, USB, L], F16, tag="h")
                            nc.scalar.activation(
                                h[:, 0:usn, :], s[:, 0:usn, :], AF.Tanh
                            )
                            for k in range(usn):
                                us = us0 + k
                                for n in range(L // 512):
                                    nc.tensor.matmul(
                                        e_ps[ds(32 * c, 32), ds(n * 512, 512)],
                                        wa32_sb[:, us, :],
                                        h[:, k, ds(n * 512, 512)],
                                        start=(us == 0),
                                        stop=(us == NUS - 1),
                                        tile_position=(0, 32 * c),
                                    )
                    p = ppool.tile([128, L], F32R)
                    nc.scalar.activation(
                        p[:], e_ps[:], AF.Exp, accum_out=sums_sb[:, ds(qb, 1)]
                    )
                    nc.vector.reciprocal(recip_sb[:, ds(qb, 1)], sums_sb[:, ds(qb, 1)])
                    at_sb = atpool.tile([128, NJC, 128], F32R)
                    at_ps = pe_at.tile([128, L], F32R)
                    for jc in range(NJC):
                        nc.tensor.transpose(
                            at_ps[:, ts(jc, 128)], p[:, ts(jc, 128)], ident_sb[:]
                        )
                    if qb == NQB - 1:
                        # ACT is done after the last exp; split the copy
                        nc.vector.tensor_copy(
                            at_sb[:, 0 : NJC // 2, :], at_ps[:, 0 : L // 2]
                        )
                        nc.scalar.copy(
                            at_sb[:, NJC // 2 :, :], at_ps[:, L // 2 :]
                        )
                    else:
                        nc.vector.tensor_copy(at_sb[:], at_ps[:])
                    v_ps = pe_v.tile([128, D], F32)
                    for jc in range(NJC):
                        nc.tensor.matmul(
                            v_ps[:],
                            at_sb[:, jc, :],
                            x_sb[:, jc, :],
                            start=(jc == 0),
                            stop=(jc == NJC - 1),
                        )
                    v_sb = vpool.tile([128, D], F32)
                    nc.vector.tensor_scalar_mul(
                        v_sb[:], v_ps[:], recip_sb[:, ds(qb, 1)]
                    )
                    nc.sync.dma_start(out_r[qb], v_sb[:])

    return nc


_NC_CACHE: dict = {}


def get_compiled_nc():
    if "nc" not in _NC_CACHE:
        nc = bacc.Bacc("TRN2", target_bir_lowering=False, debug=False)
        build_kernel(nc)
        nc.compile()
        _NC_CACHE["nc"] = nc
    return _NC_CACHE["nc"]


def make_in_maps(inputs_np, Wt, Wx, bh, Wa):
    wx4 = np.zeros((D, NUS, 128), np.float32)
    wa32 = np.zeros((NUS, 128, GQ), np.float16)
    for us in range(NUS):
        for uu in range(4):
            u = 4 * us + uu
            wx4[:, us, 32 * uu : 32 * (uu + 1)] = Wx[:, u : u + 1]
            wa32[us, 32 * uu : 32 * (uu + 1), :] = Wa[u, 0] * np.eye(GQ, dtype=np.float32)
    bh_c = bh.reshape(U, 1).astype(np.float32)
    ident = np.eye(128, dtype=np.float32)
    in_maps = []
    for c in range(NCORES):
        b, half = divmod(c, HALVES)
        in_maps.append(
            {
                "x": np.ascontiguousarray(inputs_np[b]),
                "xq": np.ascontiguousarray(inputs_np[b, half * LQ : (half + 1) * LQ]),
                "wt": Wt,
                "wx4": wx4,
                "wa32": wa32,
                "bh": bh_c,
                "ident": ident,
            }
        )
    return in_maps


def kernel(**inputs) -> np.ndarray:
    x = np.asarray(inputs["inputs"], dtype=np.float32)
    Wt = np.ascontiguousarray(np.asarray(inputs["Wt"], np.float32))
    Wx = np.ascontiguousarray(np.asarray(inputs["Wx"], np.float32))
    bh = np.asarray(inputs["bh"], np.float32)
    Wa = np.asarray(inputs["Wa"], np.float32)

    from concourse.bass_utils import run_bass_kernel_spmd

    nc = get_compiled_nc()
    in_maps = make_in_maps(x, Wt, Wx, bh, Wa)
    res = run_bass_kernel_spmd(nc, in_maps, list(range(NCORES)))
    kernel._last_results = res  # type: ignore[attr-defined]

    out = np.empty((B, L, D), np.float32)
    for c in range(NCORES):
        b, half = divmod(c, HALVES)
        out[b, half * LQ : (half + 1) * LQ] = res.results[c]["out"]
    return out


# revision 11
# speedup vs baseline: 1.0305x; 1.0043x over previous
"""Bahdanau additive-attention pooling for Trainium2 (Bass/Tile).

Reference math (per batch):
    q = x @ Wt; k = x @ Wx                                  [L, U]
    e[i,j] = sum_u Wa[u] * tanh(q[i,u] + k[j,u] + bh[u])    (+ ba, dropped --
                                                             softmax shift-inv)
    v = softmax_j(e) @ x                                    [L, D]

Sharding: 8 cores = 4 batches x 2 query-halves (data-parallel, no
collectives).  Per core: 512 queries x 1024 keys, flash-style over query
blocks of 128 so the [L, L, U] tensor h is never materialized.

Per-core layout: partitions p = 32*uu + ii, where ii indexes 32 queries of a
"group" and uu 4 of the 32 u's; u-slices us = 0..7 cover u = 4*us+uu.  Groups
are query-strided (group g = queries {16*ii + g}) so every cross-partition
data movement is a clean strided DMA; the output DMA un-permutes.

  K4[us][p, j] = k[j, 4us+uu]      PE matmul, host-replicated Wx4, fp32r
  Qb[us][p, g] = q[16ii+g, ...]+bh qT -> DRAM -> strided gather-back
  S  = K4[us] + Qb[us][:, g]       VectorE tensor_scalar; K4 and S are fp16
                                   (16-bit packed DVE mode, ~2x; halves the
                                   K4 PSUM->SBUF copy payload on ScalarE)
  H  = tanh(S)                     ScalarE, batched 4 u-slices per instr,
                                   fp16 output (the engine bottleneck:
                                   L*L*U/8 = 16.8M lanes-elems per core)
  e[32c:32c+32, :] += wa32[us].T@H PE, M=32 col-tiled at partition base 32c
                                   (fp16: full rate + legal dst partition;
                                   fp32r is full-rate but base-0 only),
                                   8 accumulating matmuls contract u
  P = exp(e)                       ScalarE on the [128q, 1024k] PSUM block,
                                   row-sums via accum_out (|e| <= ~4.5, so
                                   no max-subtraction is needed)
  aT chunks = PE transpose(P); v = sum_jc aT[jc].T @ x[jc] (fp32r); scale by
  1/rowsum on VectorE; DMA out.

Engine budget per core (model): ScalarE ~131us (86% busy - bound by the
16.8M-element tanh volume at 1 elem/cycle/lane @1.2GHz), PE ~76us,
VectorE ~51us, total ~152us.
"""

import numpy as np

import concourse.bass as bass
import concourse.mybir as mybir
import concourse.tile as tile
from concourse import bacc
from concourse.bass import ds, ts

B, L, D, U = 4, 1024, 256, 32
NCORES = 8
HALVES = 2
LQ = L // HALVES                # 512 queries per core
GQ = 32                         # queries per group
NGRP = LQ // GQ                 # 16 groups
NUS = 8                         # u-slices (4 u's each)
USB = 4                         # u-slices per tanh batch
QB = 128                        # query block (softmax granularity)
NQB = LQ // QB                  # 4
NJC = L // 128                  # 8 key chunks
NDC = D // 128                  # 2 contraction chunks

F32 = mybir.dt.float32
F32R = mybir.dt.float32r
F16 = mybir.dt.float16
AF = mybir.ActivationFunctionType


def build_kernel(nc: bass.Bass):
    x_d = nc.dram_tensor("x", [L, D], F32R, kind="ExternalInput")
    xq_d = nc.dram_tensor("xq", [LQ, D], F32R, kind="ExternalInput")
    wt_d = nc.dram_tensor("wt", [D, U], F32R, kind="ExternalInput")
    wx4_d = nc.dram_tensor("wx4", [D, NUS, 128], F32R, kind="ExternalInput")
    wa32_d = nc.dram_tensor("wa32", [NUS, 128, GQ], F16, kind="ExternalInput")
    bh_d = nc.dram_tensor("bh", [U, 1], F32, kind="ExternalInput")
    ident_d = nc.dram_tensor("ident", [128, 128], F32R, kind="ExternalInput")
    out_d = nc.dram_tensor("out", [LQ, D], F32, kind="ExternalOutput")
    qtb_d = nc.dram_tensor("qtb", [U, LQ], F32)  # scratch for the Qb gather

    with tile.TileContext(nc) as tc:
        with tc.tile_pool(name="const", bufs=1) as cpool:
            x_sb = cpool.tile([128, NJC, D], F32R)
            xq_sb = cpool.tile([128, NQB, D], F32R)
            xT_sb = cpool.tile([128, NDC, L], F32R)
            xqT_sb = cpool.tile([128, NDC, LQ], F32R)
            wt_sb = cpool.tile([128, NDC, U], F32R)
            wx4_sb = cpool.tile([128, NDC, NUS, 128], F32R)
            wa32_sb = cpool.tile([128, NUS, GQ], F16)
            bh_sb = cpool.tile([U, 1], F32)
            ident_sb = cpool.tile([128, 128], F32R)
            k4_sb = cpool.tile([128, NUS, L], F16)
            qtb_sb = cpool.tile([U, LQ], F32)
            qb_sb = cpool.tile([128, NUS, NGRP], F32)
            sums_sb = cpool.tile([128, NQB], F32)
            recip_sb = cpool.tile([128, NQB], F32)

            # small/critical DMAs first; 1MB wx4 split per-us and last
            nc.scalar.dma_start(ident_sb[:], ident_d.ap())
            nc.scalar.dma_start(bh_sb[:], bh_d.ap())
            nc.scalar.dma_start(
                wt_sb[:], wt_d.ap().rearrange("(c p) u -> p c u", p=128)
            )
            nc.scalar.dma_start(
                wa32_sb[:], wa32_d.ap().rearrange("us p m -> p us m")
            )
            nc.sync.dma_start(
                xq_sb[:], xq_d.ap().rearrange("(c p) d -> p c d", p=128)
            )
            x_r = x_d.ap().rearrange("(c p) d -> c p d", p=128)
            wx4_r = wx4_d.ap().rearrange("(c p) us m -> p c us m", p=128)
            for jc in (0, 2):
                nc.sync.dma_start(x_sb[:, jc, :], x_r[jc])
            for jc in (1, 3):
                nc.gpsimd.dma_start(x_sb[:, jc, :], x_r[jc])
            # first wx4 slices early: they gate the first K4 matmuls
            for us in (0, 1):
                nc.gpsimd.dma_start(wx4_sb[:, :, us, :], wx4_r[:, :, us, :])
            for jc in (5, 7):
                nc.gpsimd.dma_start(x_sb[:, jc, :], x_r[jc])
            for us in range(2, NUS):
                nc.gpsimd.dma_start(wx4_sb[:, :, us, :], wx4_r[:, :, us, :])

            # ---- prologue ----
            with (
                tc.tile_pool(name="ptr", bufs=3, space="PSUM") as ptr,
                tc.tile_pool(name="pk4", bufs=2, space="PSUM") as pk4,
                tc.tile_pool(name="pqt", bufs=1, space="PSUM") as pqt,
            ):
                # xq^T first: the qT -> DRAM -> gather chain is the longest
                for dc in range(NDC):
                    tr4 = ptr.tile([128, 512], F32R)
                    for jc in range(NQB):
                        nc.tensor.transpose(
                            tr4[:, ts(jc, 128)],
                            xq_sb[:, jc, ds(dc * 128, 128)],
                            ident_sb[:],
                        )
                    nc.scalar.copy(xqT_sb[:, dc, :], tr4[:])
                qt_ps = pqt.tile([U, LQ], F32)
                for dc in range(NDC):
                    nc.tensor.matmul(
                        qt_ps[:],
                        wt_sb[:, dc, :],
                        xqT_sb[:, dc, :],
                        start=(dc == 0),
                        stop=(dc == NDC - 1),
                    )
                nc.vector.tensor_scalar_add(qtb_sb[:], qt_ps[:], bh_sb[:])
                nc.sync.dma_start(qtb_d.ap(), qtb_sb[:])
                # Qb[us][32uu+ii, g] = qtb[4us+uu, 16ii+g]  (strided groups:
                # group g holds queries {16ii+g}) -> contiguous 64B runs
                qtb_r = qtb_d.ap().rearrange(
                    "(us uu) (ii g) -> uu ii us g", uu=4, g=NGRP
                )
                for uu in range(4):
                    dst = qb_sb[ds(32 * uu, GQ), :, :]
                    nc.sync.dma_start(dst, qtb_r[uu])
                # x4/x6 queued after the Qb gathers: not needed until the
                # second transpose wave, and ahead of them they delay Qb
                for jc in (4, 6):
                    nc.sync.dma_start(x_sb[:, jc, :], x_r[jc])

                # x^T: 4 chunk-transposes per PSUM tile, one copy per tile
                for n in range(L // 512):
                    for dc in range(NDC):
                        tr4 = ptr.tile([128, 512], F32R)
                        for q4 in range(4):
                            jc = 4 * n + q4
                            nc.tensor.transpose(
                                tr4[:, ts(q4, 128)],
                                x_sb[:, jc, ds(dc * 128, 128)],
                                ident_sb[:],
                            )
                        if dc == 0:
                            nc.vector.tensor_copy(
                                xT_sb[:, dc, ds(n * 512, 512)], tr4[:]
                            )
                        else:
                            nc.scalar.copy(
                                xT_sb[:, dc, ds(n * 512, 512)], tr4[:]
                            )

                # K4[us] = k^T slice-replicated, via host-replicated Wx4
                for us in range(NUS):
                    kp = pk4.tile([128, L], F32)
                    for n in range(L // 512):
                        for dc in range(NDC):
                            nc.tensor.matmul(
                                kp[:, ds(n * 512, 512)],
                                wx4_sb[:, dc, us, :],
                                xT_sb[:, dc, ds(n * 512, 512)],
                                start=(dc == 0),
                                stop=(dc == NDC - 1),
                            )
                    nc.scalar.copy(k4_sb[:, us, :], kp[:])

            # ---- main loop ----
            with (
                tc.tile_pool(name="spool", bufs=3) as spool,
                tc.tile_pool(name="hpool", bufs=3) as hpool,
                tc.tile_pool(name="ppool", bufs=2) as ppool,
                tc.tile_pool(name="atpool", bufs=2) as atpool,
                tc.tile_pool(name="vpool", bufs=2) as vpool,
                tc.tile_pool(name="pe", bufs=2, space="PSUM") as pe_e,
                tc.tile_pool(name="pat", bufs=1, space="PSUM") as pe_at,
                tc.tile_pool(name="pv", bufs=2, space="PSUM") as pe_v,
            ):
                out_r = out_d.ap().rearrange(
                    "(ii gg c) d -> gg c ii d", gg=NQB, c=4
                )
                for qb in range(NQB):
                    e_ps = pe_e.tile([128, L], F32)
                    for c in range(4):
                        g = 4 * qb + c
                        # the very last group's final batch is split 2+2 so
                        # the e-matmul stretch after the last tanh (which
                        # gates the final exp) is half as long
                        last = qb == NQB - 1 and c == 3
                        first = qb == 0 and c == 0
                        if last:
                            batches = [(0, 4), (4, 2), (6, 2)]
                        elif first:
                            # small first batch: the tanh pipeline starts as
                            # soon as 2 (not 4) S-adds complete
                            batches = [(0, 2), (2, 2), (4, 4)]
                        else:
                            batches = [(0, USB), (USB, USB)]
                        for us0, usn in batches:
                            s = spool.tile([128, USB, L], F16, tag="s")
                            for k in range(usn):
                                us = us0 + k
                                nc.vector.tensor_scalar_add(
                                    s[:, k, :],
                                    k4_sb[:, us, :],
                                    qb_sb[:, us, ds(g, 1)],
                                )
                            h = hpool.tile([128, USB, L], F16, tag="h")
                            nc.scalar.activation(
                                h[:, 0:usn, :], s[:, 0:usn, :], AF.Tanh
                            )
                            for k in range(usn):
                                us = us0 + k
                                for n in range(L // 512):
                                    nc.tensor.matmul(
                                        e_ps[ds(32 * c, 32), ds(n * 512, 512)],
                                        wa32_sb[:, us, :],
                                        h[:, k, ds(n * 512, 512)],
                                        start=(us == 0),
                                        stop=(us == NUS - 1),
                                        tile_position=(0, 32 * c),
                                    )
                    p = ppool.tile([128, L], F32R)
                    nc.scalar.activation(
                        p[:], e_ps[:], AF.Exp, accum_out=sums_sb[:, ds(qb, 1)]
                    )
                    nc.vector.reciprocal(recip_sb[:, ds(qb, 1)], sums_sb[:, ds(qb, 1)])
                    at_sb = atpool.tile([128, NJC, 128], F32R)
                    at_ps = pe_at.tile([128, L], F32R)
                    for jc in range(NJC):
                        nc.tensor.transpose(
                            at_ps[:, ts(jc, 128)], p[:, ts(jc, 128)], ident_sb[:]
                        )
                    if qb == NQB - 1:
                        # ACT is done after the last exp; split the copy
                        nc.vector.tensor_copy(
                            at_sb[:, 0 : NJC // 2, :], at_ps[:, 0 : L // 2]
                        )
                        nc.scalar.copy(
                            at_sb[:, NJC // 2 :, :], at_ps[:, L // 2 :]
                        )
                    else:
                        nc.vector.tensor_copy(at_sb[:], at_ps[:])
                    v_ps = pe_v.tile([128, D], F32)
                    for jc in range(NJC):
                        nc.tensor.matmul(
                            v_ps[:],
                            at_sb[:, jc, :],
                            x_sb[:, jc, :],
                            start=(jc == 0),
                            stop=(jc == NJC - 1),
                        )
                    v_sb = vpool.tile([128, D], F32)
                    nc.vector.tensor_scalar_mul(
                        v_sb[:], v_ps[:], recip_sb[:, ds(qb, 1)]
                    )
                    nc.sync.dma_start(out_r[qb], v_sb[:])

    return nc


_NC_CACHE: dict = {}


def get_compiled_nc():
    if "nc" not in _NC_CACHE:
        nc = bacc.Bacc("TRN2", target_bir_lowering=False, debug=False)
        build_kernel(nc)
        nc.compile()
        _NC_CACHE["nc"] = nc
    return _NC_CACHE["nc"]


def make_in_maps(inputs_np, Wt, Wx, bh, Wa):
    wx4 = np.zeros((D, NUS, 128), np.float32)
    wa32 = np.zeros((NUS, 128, GQ), np.float16)
    for us in range(NUS):
        for uu in range(4):
            u = 4 * us + uu
            wx4[:, us, 32 * uu : 32 * (uu + 1)] = Wx[:, u : u + 1]
            wa32[us, 32 * uu : 32 * (uu + 1), :] = Wa[u, 0] * np.eye(GQ, dtype=np.float32)
    bh_c = bh.reshape(U, 1).astype(np.float32)
    ident = np.eye(128, dtype=np.float32)
    in_maps = []
    for c in range(NCORES):
        b, half = divmod(c, HALVES)
        in_maps.append(
            {
                "x": np.ascontiguousarray(inputs_np[b]),
                "xq": np.ascontiguousarray(inputs_np[b, half * LQ : (half + 1) * LQ]),
                "wt": Wt,
                "wx4": wx4,
                "wa32": wa32,
                "bh": bh_c,
                "ident": ident,
            }
        )
    return in_maps


def kernel(**inputs) -> np.ndarray:
    x = np.asarray(inputs["inputs"], dtype=np.float32)
    Wt = np.ascontiguousarray(np.asarray(inputs["Wt"], np.float32))
    Wx = np.ascontiguousarray(np.asarray(inputs["Wx"], np.float32))
    bh = np.asarray(inputs["bh"], np.float32)
    Wa = np.asarray(inputs["Wa"], np.float32)

    from concourse.bass_utils import run_bass_kernel_spmd

    nc = get_compiled_nc()
    in_maps = make_in_maps(x, Wt, Wx, bh, Wa)
    res = run_bass_kernel_spmd(nc, in_maps, list(range(NCORES)))
    kernel._last_results = res  # type: ignore[attr-defined]

    out = np.empty((B, L, D), np.float32)
    for c in range(NCORES):
        b, half = divmod(c, HALVES)
        out[b, half * LQ : (half + 1) * LQ] = res.results[c]["out"]
    return out
